# revision 21
# baseline (speedup 1.0000x reference)
"""Trainium2 Bass kernel for nn_FComb_79319456023150 (dense_cnn).

Per-pixel MLP over a 96^3 volume: four 1x1x1 convs (38->32->32->32->1 channels
with relu between). z is batch-constant, so w1[:, 32:38] @ z folds into the
layer-1 bias and every layer becomes a K=32 channel GEMM.

Sharding: spatial (outermost X axis) across 8 cores, 110592 pixels each.
Weights/biases replicated.

Device layout per core: SBUF activation tiles are [128, F] = 4 pixel-blocks x
32 channels on partitions, pixels on the free dim. Each layer is computed
with a BLOCK-DIAGONAL [128, 128] weight matrix (4 copies of W^T on the
diagonal), so one full-array matmul per 512-pixel chunk applies the 32x32
GEMM to all 4 pixel blocks at once. The final layer (wl: 1x32) uses one
sparse [128, 128] weight per chunk, accumulating all chunks into a single
PSUM bank so the whole super-chunk's output is evacuated by one dense op.
Relu+bias rides the mandatory PSUM->SBUF crossing, split between ScalarE
(activation Relu w/ bias) and VectorE (fused tensor_scalar add+max) at
PSUM-bank-aligned boundaries; float32r matmuls stream at 1 col/cycle.
"""

import sys

import numpy as np

if "/opt/trn_rl_repo" not in sys.path:
    sys.path.insert(0, "/opt/trn_rl_repo")

C = 32          # channels per layer
P = 128         # SBUF/PSUM partitions
RG = 4          # pixel blocks stacked on the partition dim (128/32)
NCHUNK = 3      # 512-wide chunks per super-chunk (PSUM big tile = 3 banks)
CH = 512        # chunk width (one PSUM bank of fp32)
SCW = NCHUNK * CH                    # 1536 free-dim columns per super-chunk
VOL = 96 * 96 * 96                   # full volume
NCORES = 8
NPIX = VOL // NCORES                 # 110592 pixels per core
FREE = NPIX // RG                    # 27648 free-dim columns per core
NSC = FREE // SCW                    # 18 super-chunks per core
assert FREE % SCW == 0

# ACT/DVE split of each PSUM->SBUF relu crossing, alternating per layer so the
# two engines stay balanced (ScalarE ~1.2 col/ns, VectorE ~0.96 col/ns, both
# 1x on fp32-from-PSUM):
#   layer 0, 2: ACT gets banks 0-1 (1024 cols), DVE bank 2
#   layer 1:    ACT gets bank 0, DVE banks 1-2
_ACT_COLS = {0: 1024, 1: 512, 2: 1024}


def _build_nc(npix=NPIX, use_f32r=True):
    import concourse.mybir as mybir
    from concourse import bacc
    from concourse.tile import TileContext

    f32 = mybir.dt.float32
    f32r = mybir.dt.float32r if use_f32r else mybir.dt.float32
    Alu = mybir.AluOpType
    Act = mybir.ActivationFunctionType

    free = npix // RG
    nsc = free // SCW
    assert free % SCW == 0 and nsc >= 1

    nc = bacc.Bacc()
    fm = nc.dram_tensor("fm", [C, npix], f32r, kind="ExternalInput")
    wst = nc.dram_tensor("wst", [P, 3 * P], f32r, kind="ExternalInput")
    w4 = nc.dram_tensor("w4", [P, NCHUNK * P], f32r, kind="ExternalInput")
    bias = nc.dram_tensor("bias", [P, 4], f32, kind="ExternalInput")
    out = nc.dram_tensor("out", [npix], f32, kind="ExternalOutput")

    fm_r = fm.rearrange("c (b n) -> c b n", b=RG)       # [32, 4, free]
    out_r = out.rearrange("(m n) -> m n", m=RG)         # [4, free]

    with TileContext(nc) as tc:
        with (
            tc.tile_pool(name="const", bufs=1) as constp,
            tc.tile_pool(name="data", bufs=3) as datap,
            tc.tile_pool(name="acts", bufs=2) as actp,
            tc.tile_pool(name="outs", bufs=3) as outsp,
            tc.tile_pool(name="psb", bufs=2, space="PSUM") as psb,
            tc.tile_pool(name="pss", bufs=2, space="PSUM") as pss,
        ):
            wtile = constp.tile([P, 3 * P], f32r)
            nc.sync.dma_start(wtile, wst[:, :])
            w4tile = constp.tile([P, NCHUNK * P], f32r)
            nc.sync.dma_start(w4tile, w4[:, :])
            btile = constp.tile([P, 4], f32)
            nc.sync.dma_start(btile, bias[:, :])

            for s in range(nsc):
                x = datap.tile([P, SCW], f32r, tag="x")
                for b in range(RG):
                    nc.sync.dma_start(
                        x[32 * b:32 * b + 32, :],
                        fm_r[:, b, s * SCW:(s + 1) * SCW],
                    )

                h = x
                for layer in range(3):
                    ps = psb.tile([P, SCW], f32, tag="psA")
                    wsl = wtile[:, layer * P:(layer + 1) * P]
                    for cc in range(NCHUNK):
                        nc.tensor.matmul(
                            ps[:, cc * CH:(cc + 1) * CH],
                            wsl,
                            h[:, cc * CH:(cc + 1) * CH],
                            start=True,
                            stop=True,
                        )
                    hn = actp.tile([P, SCW], f32r, tag=f"h{layer}")
                    bcol = btile[:, layer:layer + 1]
                    acols = _ACT_COLS[layer]
                    nc.scalar.activation(
                        hn[:, :acols], ps[:, :acols], Act.Relu,
                        bias=bcol, scale=1.0,
                    )
                    nc.vector.tensor_scalar(
                        hn[:, acols:], ps[:, acols:],
                        bcol, 0.0, Alu.add, Alu.max,
                    )
                    h = hn

                # Layer 4: chunk c's [128,128] weight has wl only in columns
                # 32c..32c+4; accumulating the 3 chunk matmuls into one bank
                # leaves out[32c+m, n] = wl @ (block m of chunk c).
                ps4 = pss.tile([P, CH], f32, tag="ps4")
                for cc in range(NCHUNK):
                    nc.tensor.matmul(
                        ps4[:, :],
                        w4tile[:, cc * P:(cc + 1) * P],
                        h[:, cc * CH:(cc + 1) * CH],
                        start=(cc == 0),
                        stop=(cc == NCHUNK - 1),
                    )
                ob = outsp.tile([P, CH], f32, tag="ob")
                blcol = btile[:, 3:4]
                nc.vector.tensor_scalar(ob[:, :], ps4[:, :], blcol, None, Alu.add)
                for cc in range(NCHUNK):
                    nc.sync.dma_start(
                        out_r[:, s * SCW + cc * CH: s * SCW + (cc + 1) * CH],
                        ob[32 * cc:32 * cc + 4, :],
                    )

    # Walrus codegen cannot reliably attach semaphore waits to self-loading
    # matmuls; hoist every matmul's waits onto a PE nop inserted just before
    # it (sequencer-side wait, same semantics).
    for blk in nc.main_func.blocks:
        insts = blk.instructions
        idx = 0
        while idx < len(insts):
            inst = insts[idx]
            if isinstance(inst, mybir.InstMatmult):
                si = inst.sync_info
                if si is not None and len(si.on_wait) > 0:
                    nop = mybir.InstNoOp(
                        name=nc.get_next_instruction_name(), ins=[], outs=[]
                    )
                    nop.engine = inst.engine
                    nop.bass_nofuse = True
                    nop.sync_info = mybir.SyncInfo(on_wait=si.on_wait, on_update=[])
                    si.on_wait = []
                    nc.register_instruction(nop)
                    insts.insert(idx, nop)
                    idx += 1
            idx += 1

    for blk in nc.main_func.blocks:
        for inst in blk.instructions:
            if isinstance(inst, mybir.InstMatmult):
                si = inst.sync_info
                assert si is None or len(si.on_wait) == 0, inst.name

    nc.compile()
    return nc


def _blockdiag4(wT):
    """[32, 32] -> [128, 128] block-diagonal with 4 copies."""
    out = np.zeros((P, P), dtype=np.float32)
    for b in range(RG):
        out[32 * b:32 * b + 32, 32 * b:32 * b + 32] = wT
    return out


def _prep_host_inputs(z, w1, b1, w2, b2, w3, b3, wl, bl):
    """Fold z into the layer-1 bias and build the device weight layouts."""
    f32 = np.float32
    b1e = (b1 + w1[:, C:] @ z[0]).astype(f32)          # [32]
    wst = np.concatenate(
        [
            _blockdiag4(w1[:, :C].T),
            _blockdiag4(w2.T),
            _blockdiag4(w3.T),
        ],
        axis=1,
    ).astype(f32)                                       # [128, 384]

    w4 = np.zeros((P, NCHUNK * P), dtype=f32)
    for cc in range(NCHUNK):
        for m in range(RG):
            w4[32 * m:32 * m + 32, cc * P + 32 * cc + m] = wl[0, :]

    bias = np.zeros((P, 4), dtype=f32)
    bias[:, 0] = np.tile(b1e, RG)
    bias[:, 1] = np.tile(b2.astype(f32), RG)
    bias[:, 2] = np.tile(b3.astype(f32), RG)
    bias[:, 3] = f32(bl[0])
    return wst, w4, bias


_NC_CACHE = {}


def _run(feature_map, z, w1, b1, w2, b2, w3, b3, wl, bl, **spmd_kwargs):
    from concourse.bass_utils import run_bass_kernel_spmd

    feature_map = np.asarray(feature_map, dtype=np.float32)
    z = np.asarray(z, dtype=np.float32)
    w1, b1 = np.asarray(w1, np.float32), np.asarray(b1, np.float32)
    w2, b2 = np.asarray(w2, np.float32), np.asarray(b2, np.float32)
    w3, b3 = np.asarray(w3, np.float32), np.asarray(b3, np.float32)
    wl, bl = np.asarray(wl, np.float32), np.asarray(bl, np.float32)

    wst, w4, bias = _prep_host_inputs(z, w1, b1, w2, b2, w3, b3, wl, bl)

    fm_flat = np.ascontiguousarray(feature_map.reshape(C, VOL))
    in_maps = []
    for k in range(NCORES):
        shard = np.ascontiguousarray(fm_flat[:, k * NPIX:(k + 1) * NPIX])
        in_maps.append({"fm": shard, "wst": wst, "w4": w4, "bias": bias})

    if "nc" not in _NC_CACHE:
        _NC_CACHE["nc"] = _build_nc()
    nc = _NC_CACHE["nc"]

    res = run_bass_kernel_spmd(nc, in_maps, core_ids=list(range(NCORES)), **spmd_kwargs)
    out = np.empty((VOL,), dtype=np.float32)
    for k in range(NCORES):
        out[k * NPIX:(k + 1) * NPIX] = res.results[k]["out"]
    return out.reshape(1, 1, 96, 96, 96), res


def kernel(feature_map, z, w1, b1, w2, b2, w3, b3, wl, bl):
    out, _ = _run(feature_map, z, w1, b1, w2, b2, w3, b3, wl, bl)
    return out
